# revision 10
# baseline (speedup 1.0000x reference)
"""GraphSAGE 2-layer kernel for 8 Trainium2 NeuronCores (SPMD).

Strategy (v2):
- Nodes sorted by in-degree, padded to NPOS = 8*128*NBLK positions, dealt
  round-robin to cores in 128-lane blocks (run j = 8 cores' block j, all with
  near-equal degree). One reserved always-zero lane per core (last block,
  lane 127).
- Layer-1 aggregation: host expands x[src] into the canonical slot layout
  [128 lanes x cols x 64] bf16 (edges of dst lane p in column k); the device
  segment-sum is a plain PSUM accumulation of identity matmuls. Dense part
  per block: Y = [mean | x] @ [Wl; Wr] via one matmul with lhsT =
  [mean^T; x^T]; L2-normalize via ACT square+accum, sqrt, DVE reciprocal,
  fused scale+relu. h written as fp16 into a 256B/row layout.
- h (fp16, [*, 128] rows: cols 0:64 = h, 64:128 don't-care) is AllGathered.
- Layer-2 aggregation: edges partitioned by destination core; tokens sorted
  by (group g, window w, half h) and padded to 128 multiples per
  (g, half, w) subcell UNIFORMLY across cores (max over cores). One
  dma_gather per (g, w) fetches h[src] rows (256B each, int16 indices into
  one of 4 windows). Per 128-token chunk, a selection matrix T
  [128 tok, 512] is built ON DEVICE by DVE: T[t, col_t] = 1/deg(dst_t)
  (iota==colidx)*tval, and one PE matmul accumulates
  meanT[f, (b%4)*128+p] += h[tok, f] * T[tok, col] into a [64, 512] PSUM
  half-group tile -- mean arrives transposed with the 1/deg fold-in, no
  per-node padding, no scale or transpose steps.
- Dense part layer 2: actsT[0:64] <- meanT psum, actsT[64:128] <- PE
  transpose of own h rows; one matmul with [W2l; W2r], L2-normalize.
"""
import numpy as np
import ml_dtypes

import concourse.bass as bass
import concourse.bacc as bacc
import concourse.tile as tile
from concourse import mybir
from concourse import bass_utils

NCORES = 8
LANES = 128
BPG = 8           # blocks per group (psum accumulation granularity: 2 halves)
L1_CHUNK_COLS = 96
F_IN, F_HID, F_OUT = 64, 64, 32
BF16 = ml_dtypes.bfloat16
FP16 = np.float16
HROW = 128        # fp16 elements per h row (256B; cols 64:128 don't-care)


def _wrap_idx(flat_idx):
    """flat [n] int16 -> [128, n/16] wrapped in 16 partitions, replicated x8."""
    n = flat_idx.shape[0]
    arr = flat_idx.reshape(n // 16, 16).T
    return np.tile(arr, (8, 1)).astype(np.int16)


def _preprocess(x, edge_index, N):
    src = np.asarray(edge_index[0], dtype=np.int64)
    dst = np.asarray(edge_index[1], dtype=np.int64)
    E = src.shape[0]

    nblk = int(np.ceil((N + NCORES) / (NCORES * LANES)))
    npos = NCORES * LANES * nblk
    npc = LANES * nblk
    winrows = 2 * npc
    nwin = NCORES // 2
    ngrp = int(np.ceil(nblk / BPG))
    nb_g = [min(BPG, nblk - g * BPG) for g in range(ngrp)]

    deg = np.bincount(dst, minlength=N).astype(np.int64)

    # position space: all (run j, core c, lane p); reserved = (nblk-1, c, 127)
    res_pos = (nblk - 1) * NCORES * LANES + np.arange(NCORES) * LANES + (LANES - 1)
    avail = np.ones(npos, dtype=bool)
    avail[res_pos] = False
    avail_pos = np.flatnonzero(avail)
    nfill = npos - NCORES - N
    order = np.argsort(deg, kind="stable")
    pos2node = np.full(npos, -1, dtype=np.int64)
    pos2node[avail_pos[nfill:]] = order

    ii = np.arange(npos)
    pos_c = (ii % (NCORES * LANES)) // LANES
    pos_j = ii // (NCORES * LANES)
    pos_p = ii % LANES
    pos_row = pos_c * npc + pos_j * 128 + pos_p
    node2row = np.empty(N, dtype=np.int64)
    real = pos2node >= 0
    node2row[pos2node[real]] = pos_row[real]

    # per-run degree maxima -> L1 schedule
    degpos = np.where(real, deg[np.clip(pos2node, 0, None)], 0)
    run_deg = degpos.reshape(nblk, NCORES * LANES).max(axis=1)
    d1_g = [max(1, int(run_deg[g * BPG:g * BPG + nb_g[g]].max())) for g in range(ngrp)]

    # CSR by dst (for L1 slot expansion)
    eord = np.argsort(dst, kind="stable")
    s_by_dst = src[eord]
    indptr = np.zeros(N + 1, dtype=np.int64)
    indptr[1:] = np.cumsum(deg)

    xbf = np.asarray(x, dtype=np.float32).astype(BF16)
    xf = np.asarray(x, dtype=np.float32)

    # node id at (c, j, p)
    node_cjp = np.full((NCORES, nblk, LANES), -1, dtype=np.int64)
    node_cjp[pos_c[real], pos_j[real], pos_p[real]] = pos2node[real]

    deg_cjp = np.where(node_cjp >= 0, deg[np.clip(node_cjp, 0, None)], 0)
    ip_cjp = np.where(node_cjp >= 0, indptr[np.clip(node_cjp, 0, None)], 0)

    # ---- L1 slots expansion + schedule ----
    tot1 = sum(d1_g[g] * nb_g[g] for g in range(ngrp))
    slots1 = [np.zeros((128, tot1, F_IN), dtype=BF16) for _ in range(NCORES)]
    l1_sched = []  # per group: (col_offset, d1, nb)
    cofs = 0
    for g in range(ngrp):
        d1, nb = d1_g[g], nb_g[g]
        l1_sched.append((cofs, d1, nb))
        for b in range(nb):
            j = g * BPG + b
            for c in range(NCORES):
                db = deg_cjp[c, j]
                base = ip_cjp[c, j][:, None] + np.arange(d1)[None, :]
                valid = np.arange(d1)[None, :] < db[:, None]
                sidx = np.where(valid, s_by_dst[np.clip(base, 0, E - 1)], 0)
                vals = np.where(valid[:, :, None], xbf[sidx], BF16(0))
                slots1[c][:, cofs + b + np.arange(d1) * nb, :] = vals
        cofs += d1 * nb
    assert cofs == tot1

    # ---- L2 token streams (v2: T-matrix, no per-node padding) ----
    # hgat layout: j < J0 first (AllGather #0), then j >= J0 (AllGather #1)
    J0 = 48
    B0 = NCORES * J0 * 128 // 2
    B1 = NCORES * (nblk - J0) * 128 // 2
    assert B0 <= 32767 and B1 <= 32767
    winbounds = [0, B0, 2 * B0, 2 * B0 + B1, 2 * B0 + 2 * B1]
    srow = node2row[src]
    drow = node2row[dst]
    c_s = srow // npc
    rr_s = srow % npc
    j_s = rr_s // 128
    p_s = rr_s % 128
    hrow = np.where(j_s < J0,
                    c_s * J0 * 128 + j_s * 128 + p_s,
                    2 * B0 + c_s * (nblk - J0) * 128 + (j_s - J0) * 128 + p_s)
    dcore = drow // npc
    rr = drow % npc
    ej = rr // 128
    ep = rr % 128
    eg = ej // BPG
    eb = ej % BPG
    eh = eb // 4
    ecol = (eb % 4) * 128 + ep              # column in the [64, 512] half tile
    ew = np.where(hrow < 2 * B0, hrow // B0, 2 + (hrow - 2 * B0) // B1)
    erloc = np.where(hrow < 2 * B0, hrow % B0, (hrow - 2 * B0) % B1)

    ncell = ngrp * nwin * 2
    cellid = (eg * nwin + ew) * 2 + eh       # stream order: g, w, h
    cnt = np.zeros((NCORES, ncell), dtype=np.int64)
    np.add.at(cnt, (dcore, cellid), 1)
    maxcnt = cnt.max(axis=0)
    ntok_cell = ((maxcnt + 127) // 128 * 128).astype(np.int64)
    nch_cell = ntok_cell // 128
    ofs_tok = np.zeros(ncell + 1, dtype=np.int64)
    ofs_tok[1:] = np.cumsum(ntok_cell)
    tot_tok = int(ofs_tok[-1])
    nch_total = tot_tok // 128
    ofs_ch = ofs_tok // 128

    idx2 = []
    colv = []
    for c in range(NCORES):
        m = dcore == c
        so = np.lexsort((ep[m], eb[m], cellid[m]))
        ck = cellid[m][so]
        first = np.searchsorted(ck, np.arange(ncell), side="left")
        pos_in_cell = np.arange(len(ck)) - first[ck]
        gpos = ofs_tok[ck] + pos_in_cell
        rl = np.zeros(tot_tok, dtype=np.int64)
        cv = np.full(tot_tok, 999.0, dtype=np.float32)  # pad: no iota match
        rl[gpos] = erloc[m][so]
        cv[gpos] = ecol[m][so]
        idx2.append(_wrap_idx(rl.astype(np.int16)))
        colv.append(cv.reshape(nch_total, 128).T.astype(np.float32))

    # per-(g,w) gather extents + per-(g,h) chunk totals
    l2_gw = []   # [g][w] = (ntok, [(h, nch), ...])
    tot_gh = np.zeros((ngrp, 2), dtype=np.int64)
    maxtok_gw = 0
    maxch_gw = 0
    for g in range(ngrp):
        row = []
        for w in range(nwin):
            cl = (g * nwin + w) * 2
            ntok = int(ntok_cell[cl] + ntok_cell[cl + 1])
            hs = [(h, int(nch_cell[cl + h])) for h in range(2)
                  if nch_cell[cl + h] > 0]
            row.append((ntok, hs))
            maxtok_gw = max(maxtok_gw, ntok)
            maxch_gw = max(maxch_gw, ntok // 128)
            for h in range(2):
                tot_gh[g, h] += int(nch_cell[cl + h])
        l2_gw.append(row)

    # ---- dense inputs (L1 + L2 meanT scale) ----
    xT = np.zeros((NCORES, nblk, F_IN, 128), dtype=np.float32)
    invc = np.zeros((NCORES, 128, nblk), dtype=np.float32)
    invcT = np.zeros((NCORES, nblk, 64, 128), dtype=np.float32)
    for c in range(NCORES):
        nodes = node_cjp[c]
        ok = nodes >= 0
        xv = np.where(ok[:, :, None], xf[np.clip(nodes, 0, None)], 0.0)
        xT[c] = xv.transpose(0, 2, 1)
        invc[c] = np.where(ok, 1.0 / np.maximum(deg_cjp[c], 1), 0.0).T
        invcT[c] = np.broadcast_to(invc[c].T[:, None, :], (nblk, 64, 128))

    meta = dict(nblk=nblk, npos=npos, npc=npc, winbounds=winbounds, J0=J0,
                nwin=nwin, ngrp=ngrp, nb_g=nb_g, d1_g=d1_g, l1_sched=l1_sched,
                tot1=tot1, l2_gw=l2_gw, tot_gh=tot_gh, tot_tok=tot_tok,
                nch_total=nch_total, maxtok_gw=maxtok_gw, maxch_gw=maxch_gw,
                node2row=node2row)
    per_core = dict(slots1=[s.reshape(128, tot1 * F_IN) for s in slots1],
                    idx2=idx2, colv=colv, xT=xT, invc=invc, invcT=invcT)
    return meta, per_core


def _build(meta):
    nblk, npc, npos = meta["nblk"], meta["npc"], meta["npos"]
    winbounds, nwin, ngrp = meta["winbounds"], meta["nwin"], meta["ngrp"]
    J0 = meta["J0"]
    nb_g, l1_sched, tot1 = meta["nb_g"], meta["l1_sched"], meta["tot1"]
    l2_gw, tot_gh = meta["l2_gw"], meta["tot_gh"]
    tot_tok, nch_total = meta["tot_tok"], meta["nch_total"]
    maxtok_gw, maxch_gw = meta["maxtok_gw"], meta["maxch_gw"]

    nc = bacc.Bacc("TRN2", target_bir_lowering=False, debug=False,
                   num_devices=NCORES)
    slots1 = nc.dram_tensor("slots1", [128, tot1 * F_IN], mybir.dt.bfloat16,
                            kind="ExternalInput")
    idx2 = nc.dram_tensor("idx2", [128, tot_tok // 16], mybir.dt.int16,
                          kind="ExternalInput")
    colv_d = nc.dram_tensor("colv", [128, nch_total], mybir.dt.float32,
                            kind="ExternalInput")
    xT = nc.dram_tensor("xT", [nblk, F_IN, 128], mybir.dt.float32,
                        kind="ExternalInput")
    invc_d = nc.dram_tensor("invc", [128, nblk], mybir.dt.float32,
                            kind="ExternalInput")
    invcT_d = nc.dram_tensor("invcT", [nblk, 64, 128], mybir.dt.float32,
                             kind="ExternalInput")
    w1s = nc.dram_tensor("w1s", [128, F_HID], mybir.dt.float32,
                         kind="ExternalInput")
    w2s = nc.dram_tensor("w2s", [128, F_OUT], mybir.dt.float32,
                         kind="ExternalInput")
    b1t = nc.dram_tensor("b1t", [128, F_HID], mybir.dt.float32,
                         kind="ExternalInput")
    b2t = nc.dram_tensor("b2t", [128, F_OUT], mybir.dt.float32,
                         kind="ExternalInput")
    identf = nc.dram_tensor("identf", [128, 128], mybir.dt.float32,
                            kind="ExternalInput")
    identb = nc.dram_tensor("identb", [128, 128], mybir.dt.bfloat16,
                            kind="ExternalInput")
    identh = nc.dram_tensor("identh", [128, 128], mybir.dt.float16,
                            kind="ExternalInput")
    iota_d = nc.dram_tensor("iota", [128, 512], mybir.dt.float16,
                            kind="ExternalInput")
    out_d = nc.dram_tensor("out", [npc, F_OUT], mybir.dt.float32,
                           kind="ExternalOutput")

    with tile.TileContext(nc) as tc:
        with (
            tc.tile_pool(name="const", bufs=1) as cp,
            tc.tile_pool(name="slots", bufs=3) as sp,
            tc.tile_pool(name="gath", bufs=3) as gp,
            tc.tile_pool(name="idxp", bufs=3) as ixp,
            tc.tile_pool(name="tmat", bufs=4) as tp,
            tc.tile_pool(name="blk", bufs=3) as bp,
            tc.tile_pool(name="psT", bufs=2, space="PSUM") as psT,
            tc.tile_pool(name="psD", bufs=2, space="PSUM") as psD,
            tc.tile_pool(name="dram", bufs=1, space="DRAM") as dp,
        ):
            idf = cp.tile([128, 128], mybir.dt.float32, tag="idf")
            nc.sync.dma_start(idf[:], identf[:])
            idb = cp.tile([128, 128], mybir.dt.bfloat16, tag="idb")
            nc.sync.dma_start(idb[:], identb[:])
            idh = cp.tile([128, 128], mybir.dt.float16, tag="idh")
            nc.sync.dma_start(idh[:], identh[:])
            iot = cp.tile([128, 512], mybir.dt.float16, tag="iot")
            nc.sync.dma_start(iot[:], iota_d[:])
            w1 = cp.tile([128, F_HID], mybir.dt.float32, tag="w1")
            nc.sync.dma_start(w1[:], w1s[:])
            w2 = cp.tile([128, F_OUT], mybir.dt.float32, tag="w2")
            nc.sync.dma_start(w2[:], w2s[:])
            bt1 = cp.tile([128, F_HID], mybir.dt.float32, tag="bt1")
            nc.sync.dma_start(bt1[:], b1t[:])
            bt2 = cp.tile([128, F_OUT], mybir.dt.float32, tag="bt2")
            nc.sync.dma_start(bt2[:], b2t[:])
            icn = cp.tile([128, nblk], mybir.dt.float32, tag="icn")
            nc.sync.dma_start(icn[:], invc_d[:])
            epst = cp.tile([128, 1], mybir.dt.float32, tag="epst")
            nc.vector.memset(epst[:], 1e-24)

            hshard = dp.tile([npc, HROW], mybir.dt.float16)
            hgat = dp.tile([npos, HROW], mybir.dt.float16)

            def norm_to(y, fdim, relu, dst_slice):
                """L2-normalize rows of y [128, fdim] (+opt relu) -> SBUF."""
                sq = bp.tile([128, F_HID], mybir.dt.float32, tag="sq")
                ss = bp.tile([128, 1], mybir.dt.float32, tag="ss")
                nc.scalar.activation(out=sq[:, :fdim], in_=y[:],
                                     func=mybir.ActivationFunctionType.Square,
                                     accum_out=ss[:])
                s = bp.tile([128, 1], mybir.dt.float32, tag="s")
                nc.scalar.activation(out=s[:], in_=ss[:],
                                     func=mybir.ActivationFunctionType.Sqrt,
                                     bias=epst[:])
                rv = bp.tile([128, 1], mybir.dt.float32, tag="rv")
                nc.vector.reciprocal(rv[:], s[:])
                if relu:
                    nc.vector.tensor_scalar(out=dst_slice, in0=y[:],
                                            scalar1=rv[:], scalar2=0.0,
                                            op0=mybir.AluOpType.mult,
                                            op1=mybir.AluOpType.max)
                else:
                    nc.vector.tensor_scalar_mul(dst_slice, y[:], rv[:])

            # ---------------- layer 1 (+ split all-gather) ----------------
            psA_ctx = tc.tile_pool(name="psA", bufs=2, space="PSUM")
            psA = psA_ctx.__enter__()
            for g in range(ngrp):
                cofs, d1, nb = l1_sched[g]
                pa = psA.tile([128, 512], mybir.dt.float32, space="PSUM", tag="pa")
                k0 = 0
                first = True
                while k0 < d1:
                    nk = min(max(1, L1_CHUNK_COLS // nb), d1 - k0)
                    ncols = nk * nb
                    st = sp.tile([128, L1_CHUNK_COLS * F_IN], mybir.dt.bfloat16,
                                 tag="st")
                    nc.sync.dma_start(
                        st[:, :ncols * F_IN],
                        slots1[:, (cofs + k0 * nb) * F_IN:
                               (cofs + (k0 + nk) * nb) * F_IN])
                    for k in range(nk):
                        last = (k0 + k == d1 - 1)
                        nc.tensor.matmul(
                            out=pa[:, :nb * F_IN],
                            lhsT=idb[:],
                            rhs=st[:, k * nb * F_IN:(k + 1) * nb * F_IN],
                            start=first, stop=last)
                        first = False
                    k0 += nk
                hg = bp.tile([128, BPG * F_HID], mybir.dt.float16, tag="hg")
                for b in range(nb):
                    j = g * BPG + b
                    mean = bp.tile([128, F_IN], mybir.dt.float32, tag="mean")
                    nc.vector.tensor_scalar_mul(
                        mean[:], pa[:, b * F_IN:(b + 1) * F_IN],
                        icn[:, j:j + 1])
                    mT = psT.tile([64, 128], mybir.dt.float32, space="PSUM",
                                  tag="mT")
                    nc.tensor.transpose(out=mT[:], in_=mean[:], identity=idf[:])
                    actsT = bp.tile([128, 128], mybir.dt.float32, tag="actsT")
                    nc.vector.tensor_copy(actsT[0:64, :], mT[:])
                    nc.sync.dma_start(actsT[64:128, :], xT[j, :, :])
                    pd = psD.tile([128, F_HID], mybir.dt.float32, space="PSUM",
                                  tag="pd")
                    nc.tensor.matmul(out=pd[:], lhsT=actsT[:], rhs=w1[:],
                                     start=True, stop=True)
                    y = bp.tile([128, F_HID], mybir.dt.float32, tag="y")
                    nc.vector.tensor_tensor(out=y[:], in0=pd[:], in1=bt1[:],
                                            op=mybir.AluOpType.add)
                    norm_to(y, F_HID, True, hg[:, b * F_HID:(b + 1) * F_HID])
                dst = hshard[g * BPG * 128:(g * BPG + nb) * 128, 0:F_HID]
                nc.sync.dma_start(
                    dst.rearrange("(b p) f -> p b f", b=nb),
                    hg[:, :nb * F_HID].rearrange("p (b f) -> p b f", b=nb))
                if g * BPG + nb == J0:
                    # first-half h complete: overlap AllGather #0 with L1 tail
                    nc.gpsimd.collective_compute(
                        "AllGather", mybir.AluOpType.bypass,
                        replica_groups=[list(range(NCORES))],
                        ins=[hshard[0:J0 * 128, :]],
                        outs=[hgat[0:NCORES * J0 * 128, :]])
            psA_ctx.__exit__(None, None, None)

            nc.gpsimd.collective_compute(
                "AllGather", mybir.AluOpType.bypass,
                replica_groups=[list(range(NCORES))],
                ins=[hshard[J0 * 128:, :]],
                outs=[hgat[NCORES * J0 * 128:, :]])

            # ---------------- layer 2 ----------------
            psM_ctx = tc.tile_pool(name="psM", bufs=2, space="PSUM")
            psM = psM_ctx.__enter__()
            iofs = 0
            chofs = 0
            nTgen = 0
            seen = np.zeros((ngrp, 2), dtype=np.int64)
            for g in range(ngrp):
                nb = nb_g[g]
                psml = psM.tile([64, 512], mybir.dt.float32, space="PSUM",
                                tag="psml")
                psmh = psM.tile([64, 512], mybir.dt.float32, space="PSUM",
                                tag="psmh")
                psm = [psml, psmh]
                for w in range(nwin):
                    ntok, hs = l2_gw[g][w]
                    if ntok == 0:
                        continue
                    nch = ntok // 128
                    it = ixp.tile([128, maxtok_gw // 16], mybir.dt.int16,
                                  tag="it")
                    nc.sync.dma_start(it[:, :ntok // 16],
                                      idx2[:, iofs:iofs + ntok // 16])
                    gt = gp.tile([128, maxch_gw * HROW], mybir.dt.float16,
                                 tag="gt")
                    gt3 = gt[:, :nch * HROW].rearrange("p (c f) -> p c f",
                                                       c=nch)
                    nc.gpsimd.dma_gather(
                        out_ap=gt3,
                        in_ap=hgat[winbounds[w]:winbounds[w + 1], :],
                        idxs_ap=it[:, :ntok // 16],
                        num_idxs=ntok,
                        num_idxs_reg=ntok,
                        elem_size=HROW,
                        single_packet=False)
                    cv = ixp.tile([128, maxch_gw], mybir.dt.float32, tag="cv")
                    nc.sync.dma_start(cv[:, :nch], colv_d[:, chofs:chofs + nch])
                    k = 0
                    for h, nchh in hs:
                        for _ in range(nchh):
                            tt = tp.tile([128, 512], mybir.dt.float16,
                                         tag="tt")
                            if nTgen % 8 < 5:
                                nc.vector.tensor_scalar(
                                    out=tt[:], in0=iot[:],
                                    scalar1=cv[:, k:k + 1],
                                    scalar2=None,
                                    op0=mybir.AluOpType.is_equal)
                            else:
                                td = tp.tile([128, 512], mybir.dt.float16,
                                             tag="td")
                                nc.scalar.activation(
                                    out=td[:], in_=iot[:],
                                    func=mybir.ActivationFunctionType.Abs,
                                    bias=cv[:, k:k + 1], scale=-1.0)
                                nc.scalar.activation(
                                    out=tt[:], in_=td[:],
                                    func=mybir.ActivationFunctionType.Relu,
                                    bias=1.0, scale=-1.0)
                            nTgen += 1
                            st_ = seen[g, h] == 0
                            seen[g, h] += 1
                            sp_ = seen[g, h] == tot_gh[g, h]
                            nc.tensor.matmul(
                                out=psm[h][:],
                                lhsT=gt3[:, k, 0:F_HID],
                                rhs=tt[:],
                                start=bool(st_), stop=bool(sp_))
                            k += 1
                    iofs += ntok // 16
                    chofs += nch
                hr8 = bp.tile([128, BPG * F_HID], mybir.dt.float16, tag="hr8")
                hsrc = hshard[g * BPG * 128:(g * BPG + nb) * 128, 0:F_HID]
                nc.sync.dma_start(
                    hr8[:, :nb * F_HID].rearrange("p (b f) -> p b f", b=nb),
                    hsrc.rearrange("(b p) f -> p b f", b=nb))
                ivg = bp.tile([64, BPG * 128], mybir.dt.float32, tag="ivg")
                nc.sync.dma_start(
                    ivg[:, :nb * 128].rearrange("q (b l) -> q b l", b=nb),
                    invcT_d[g * BPG:g * BPG + nb].rearrange("b q l -> q b l"))
                og = bp.tile([128, BPG * F_OUT], mybir.dt.float32, tag="og")
                for b in range(nb):
                    j = g * BPG + b
                    h = b // 4
                    actsT = bp.tile([128, 128], mybir.dt.float32, tag="actsT")
                    if tot_gh[g, h] == 0:
                        nc.vector.memset(actsT[0:64, :], 0.0)
                    else:
                        nc.vector.tensor_tensor(
                            out=actsT[0:64, :],
                            in0=psm[h][:, (b % 4) * 128:(b % 4 + 1) * 128],
                            in1=ivg[:, b * 128:(b + 1) * 128],
                            op=mybir.AluOpType.mult)
                    hT = psT.tile([64, 128], mybir.dt.float16, space="PSUM",
                                  tag="mT")
                    nc.tensor.transpose(
                        out=hT[:], in_=hr8[:, b * F_HID:(b + 1) * F_HID],
                        identity=idh[:])
                    nc.scalar.copy(actsT[64:128, :], hT[:])
                    pd = psD.tile([128, F_HID], mybir.dt.float32, space="PSUM",
                                  tag="pd")
                    nc.tensor.matmul(out=pd[:, :F_OUT], lhsT=actsT[:],
                                     rhs=w2[:], start=True, stop=True)
                    y = bp.tile([128, F_HID], mybir.dt.float32, tag="y")
                    nc.vector.tensor_tensor(out=y[:, :F_OUT],
                                            in0=pd[:, :F_OUT], in1=bt2[:],
                                            op=mybir.AluOpType.add)
                    norm_to(y[:, :F_OUT], F_OUT, False,
                            og[:, b * F_OUT:(b + 1) * F_OUT])
                odst = out_d[g * BPG * 128:(g * BPG + nb) * 128, :]
                nc.sync.dma_start(
                    odst.rearrange("(b p) f -> p b f", b=nb),
                    og[:, :nb * F_OUT].rearrange("p (b f) -> p b f", b=nb))
            psM_ctx.__exit__(None, None, None)
    nc.compile()
    return nc


def make_in_maps(meta, per_core, W1l, b1, W1r, W2l, b2, W2r):
    w1s = np.concatenate([np.asarray(W1l, np.float32),
                          np.asarray(W1r, np.float32)], axis=0)
    w2s = np.concatenate([np.asarray(W2l, np.float32),
                          np.asarray(W2r, np.float32)], axis=0)
    b1t = np.tile(np.asarray(b1, np.float32)[None, :], (128, 1))
    b2t = np.tile(np.asarray(b2, np.float32)[None, :], (128, 1))
    identf = np.eye(128, dtype=np.float32)
    iota = np.tile(np.arange(512, dtype=FP16)[None, :], (128, 1))
    in_maps = []
    for c in range(NCORES):
        in_maps.append(dict(
            slots1=per_core["slots1"][c],
            idx2=per_core["idx2"][c],
            colv=per_core["colv"][c],
            xT=per_core["xT"][c],
            invc=per_core["invc"][c],
            invcT=per_core["invcT"][c],
            w1s=w1s, w2s=w2s, b1t=b1t, b2t=b2t,
            identf=identf, identb=identf.astype(BF16),
            identh=identf.astype(FP16), iota=iota,
        ))
    return in_maps


def kernel(x, edge_index, W1l, b1, W1r, W2l, b2, W2r):
    x = np.asarray(x, dtype=np.float32)
    N = x.shape[0]
    meta, per_core = _preprocess(x, edge_index, N)
    nc = _build(meta)
    in_maps = make_in_maps(meta, per_core, W1l, b1, W1r, W2l, b2, W2r)
    res = bass_utils.run_bass_kernel_spmd(nc, in_maps,
                                          core_ids=list(range(NCORES)))
    outs = np.concatenate([res.results[c]["out"] for c in range(NCORES)],
                          axis=0)
    full = outs[meta["node2row"]]
    return full.astype(np.float32)


if __name__ == "__main__":
    rng = np.random.default_rng(0)
    N, E = 100000, 1000000
    x = rng.standard_normal((N, 64), dtype=np.float32)
    ei = rng.integers(0, N, size=(2, E)).astype(np.int64)
    out = kernel(x=x, edge_index=ei,
                 W1l=rng.standard_normal((64, 64), dtype=np.float32) / 8,
                 b1=np.zeros(64, np.float32),
                 W1r=rng.standard_normal((64, 64), dtype=np.float32) / 8,
                 W2l=rng.standard_normal((64, 32), dtype=np.float32) / 8,
                 b2=np.zeros(32, np.float32),
                 W2r=rng.standard_normal((64, 32), dtype=np.float32) / 8)
    print(out.shape, out.dtype)
